# revision 4
# baseline (speedup 1.0000x reference)
"""GQA attention block (B=2,S=2048,D=4096,H=32,KVH=8,HD=128) on 8 trn2 cores.

Sharding: core c -> batch b=c//4, head-group g=c%4 (8 q heads, 2 kv heads per
core).  Each core computes QKV projections + RoPE + causal attention + its
slice of the output projection; the host sums the 4 partial outputs per batch.

Attention uses a max-free softmax: p = exp(s - B) with a data-calibrated
constant bias B (exact softmax ratio invariance), so the per-row max/stats
pass is eliminated.  The denominator l = sum_k p is accumulated on the PE
with a broadcast ones-matmul, inverted as exp(-ln l) on the Act engine, and
folded into the o^T eviction on the vector engine.
"""

import numpy as np
import ml_dtypes

import concourse.bass as bass
import concourse.tile as tile
import concourse.mybir as mybir
from concourse import bacc
from concourse.bass_utils import run_bass_kernel_spmd
from concourse.masks import make_identity

F32 = mybir.dt.float32
F32R = mybir.dt.float32r
BF16 = mybir.dt.bfloat16
AX = mybir.AxisListType
AF = mybir.ActivationFunctionType

B, S, D = 2, 2048, 4096
H, KVH, HD = 32, 8, 128
N_REP = H // KVH
N_CORES = 8
NH = 8            # q heads per core
NKV = 2           # kv heads per core
DCH = D // 128    # contraction chunks
NQT = S // 128    # q tiles
NKT = S // 128    # k tiles
NCT = NH + 2 * NKV  # projection col-tiles: 8 q, 2 k, 2 v
QC = 512          # attention q-chunk width
NQC = S // QC


def _build(causal: bool, repeat: int = 1, bias: float = 16.0):
    nc = bacc.Bacc(None, target_bir_lowering=False, debug=False)

    TPW = 512                 # qkv token-pass width
    NTPW = S // TPW

    xT = nc.dram_tensor("xT", [D, S], BF16, kind="ExternalInput")
    # weights staged as per-col-tile slabs: slab[c*128+p, d*128+j] = w[d*128+p, c*128+j]
    # -> one contiguous-line DMA loads all 32 contraction tiles of col-tile c
    wq = nc.dram_tensor("wq", [NH * 128, DCH * 128], BF16, kind="ExternalInput")
    wk = nc.dram_tensor("wk", [NKV * 128, DCH * 128], BF16, kind="ExternalInput")
    wv = nc.dram_tensor("wv", [NKV * 128, DCH * 128], BF16, kind="ExternalInput")
    wo = nc.dram_tensor("wo", [NH * HD, D], BF16, kind="ExternalInput")
    cos_rep = nc.dram_tensor("cos_rep", [128, S], F32, kind="ExternalInput")
    sin_rep = nc.dram_tensor("sin_rep", [128, S], F32, kind="ExternalInput")
    if causal:
        mask_t_in = nc.dram_tensor("mask_diag_t", [128, S], BF16, kind="ExternalInput")
    else:
        mask_t_in = nc.dram_tensor("mask_full_t", [S, S], BF16, kind="ExternalInput")
    out = nc.dram_tensor("out", [S, D], BF16, kind="ExternalOutput")

    with tile.TileContext(nc) as tc:
        with (
            tc.tile_pool(name="const", bufs=1) as constp,
        ):
            ident_f = constp.tile([128, 128], F32, tag="ident_f")
            make_identity(nc, ident_f[:])
            ident_r = constp.tile([128, 128], F32R, tag="ident_r")
            nc.vector.tensor_copy(ident_r[:], ident_f[:])
            ident_bf = constp.tile([128, 128], BF16, tag="ident_bf")
            nc.vector.tensor_copy(ident_bf[:], ident_f[:])
            ones_bf = constp.tile([128, 128], BF16, tag="ones_bf")
            nc.vector.memset(ones_bf[:], 1.0)
            nbias = constp.tile([128, 1], F32, tag="nbias")
            nc.vector.memset(nbias[:], -float(bias))

            for _rep in range(repeat):
                with (
                    tc.tile_pool(name="acts", bufs=1) as acts,
                ):
                    qT = [acts.tile([128, S], F32R, tag=f"qT{h}", name=f"qT{h}")
                          for h in range(NH)]
                    kT = [acts.tile([128, S], F32R, tag=f"kT{k}", name=f"kT{k}")
                          for k in range(NKV)]
                    v_sb = [acts.tile([128, S], BF16, tag=f"v{k}", name=f"v{k}")
                            for k in range(NKV)]

                    # ---------- Phase 1: QKV projection + RoPE ----------
                    with (
                        tc.tile_pool(name="xq", bufs=DCH + 16) as xqp,
                        tc.tile_pool(name="wslab", bufs=4) as wslabp,
                        tc.tile_pool(name="rope", bufs=2) as ropep,
                        tc.tile_pool(name="trig", bufs=1) as trigp,
                        tc.tile_pool(name="vtmp", bufs=2) as vtmpp,
                        tc.tile_pool(name="ps_qkv", bufs=2, space="PSUM") as psq,
                        tc.tile_pool(name="ps_v", bufs=2, space="PSUM") as psv,
                    ):
                        def rope_evict(ps, dest, cos_t, sin_t):
                            qc = ropep.tile([128, TPW], F32, tag="qc", name="qc")
                            qs = ropep.tile([128, TPW], F32, tag="qs", name="qs")
                            qsw = ropep.tile([128, TPW], F32, tag="qsw", name="qsw")
                            nc.vector.tensor_mul(qc[:], ps[:], cos_t[:])
                            nc.vector.tensor_mul(qs[:], ps[:], sin_t[:])
                            nc.scalar.dma_start(qsw[0:64, :], qs[64:128, :])
                            nc.scalar.dma_start(qsw[64:128, :], qs[0:64, :])
                            nc.vector.tensor_sub(dest[0:64, :], qc[0:64, :], qsw[0:64, :])
                            nc.vector.tensor_add(dest[64:128, :], qc[64:128, :], qsw[64:128, :])

                        cos_full = trigp.tile([128, S], F32, tag="cos", name="cos_full")
                        sin_full = trigp.tile([128, S], F32, tag="sin", name="sin_full")
                        nc.sync.dma_start(cos_full[:], cos_rep.ap())
                        nc.sync.dma_start(sin_full[:], sin_rep.ap())
                        for tp in range(NTPW):
                            t0 = tp * TPW
                            cos_t = cos_full[:, t0:t0 + TPW]
                            sin_t = sin_full[:, t0:t0 + TPW]
                            xt = []
                            for d in range(DCH):
                                xd = xqp.tile([128, TPW], BF16, tag="x", name="xd")
                                nc.scalar.dma_start(
                                    xd[:], xT.ap()[d * 128:(d + 1) * 128, t0:t0 + TPW])
                                xt.append(xd)

                            for ct in list(range(NH, NCT)) + list(range(NH)):
                                if ct < NH:
                                    wsrc, col, is_q, is_rope = wq, ct, True, True
                                elif ct < NH + NKV:
                                    k = ct - NH
                                    wsrc, col, is_q, is_rope = wk, k, False, True
                                else:
                                    k = ct - NH - NKV
                                    wsrc, col, is_q, is_rope = wv, k, False, False

                                slab = wslabp.tile([128, DCH * 128], BF16,
                                                   tag="wslab", name="slab")
                                nc.sync.dma_start(
                                    slab[:], wsrc.ap()[col * 128:(col + 1) * 128, :])
                                ps = psq.tile([128, TPW], F32, tag="ps", name="ps")
                                for d in range(DCH):
                                    nc.tensor.matmul(ps[:], slab[:, d * 128:(d + 1) * 128],
                                                     xt[d][:],
                                                     start=(d == 0), stop=(d == DCH - 1))

                                if is_q:
                                    rope_evict(ps, qT[ct][:, t0:t0 + TPW], cos_t, sin_t)
                                elif is_rope:
                                    rope_evict(ps, kT[k][:, t0:t0 + TPW], cos_t, sin_t)
                                else:
                                    vt = vtmpp.tile([128, TPW], F32R, tag="vt", name="vt")
                                    nc.scalar.copy(vt[:], ps[:])
                                    for kk in range(TPW // 128):
                                        tt = (t0 + kk * 128) // 128
                                        pv = psv.tile([128, 128], F32R, tag="pv", name="pv")
                                        nc.tensor.transpose(
                                            pv[:], vt[:, kk * 128:(kk + 1) * 128], ident_r[:])
                                        nc.scalar.copy(
                                            v_sb[k][:, tt * 128:(tt + 1) * 128], pv[:])

                    # ---------- Phases 2+3 share the oT pool ----------
                    with tc.tile_pool(name="oTp", bufs=1) as oTp:
                        oT_sb = [oTp.tile([128, S], BF16, tag=f"oT{h}",
                                          name=f"oT{h}") for h in range(NH)]

                        # ---------- Phase 2: attention ----------
                        with (
                            tc.tile_pool(name="maskp", bufs=1 if causal else 4) as maskp,
                            tc.tile_pool(name="ptp", bufs=3) as ptp,
                            tc.tile_pool(name="scrp", bufs=2) as scrp,
                            tc.tile_pool(name="linvp", bufs=2) as linvp,
                            tc.tile_pool(name="ps_s", bufs=3, space="PSUM") as pss,
                            tc.tile_pool(name="ps_ot", bufs=2, space="PSUM") as psot,
                            tc.tile_pool(name="ps_l", bufs=2, space="PSUM") as psl,
                        ):
                            if causal:
                                mask_t_sb = maskp.tile([128, S], BF16, tag="mask_t")
                                nc.sync.dma_start(mask_t_sb[:], mask_t_in.ap())

                            # flat tile list: (h, qc, ki, q0, cw, first, last)
                            tiles = []
                            for h in range(NH):
                                for qc in range(NQC):
                                    qc0 = qc * QC
                                    nki = (qc0 + QC) // 128 if causal else NKT
                                    for ki in range(nki):
                                        q0 = max(ki * 128, qc0) if causal else qc0
                                        tiles.append(
                                            (h, qc, ki, q0, qc0 + QC - q0,
                                             ki == 0, ki == nki - 1))
                                    # (h, qc) boundary marker
                            n_tiles = len(tiles)
                            sp_of = [None] * n_tiles
                            ctx = {}

                            def emit_score(t):
                                h, qc, ki, q0, cw, first, last = tiles[t]
                                kv = h // N_REP
                                sp = pss.tile([128, QC], F32, tag="sp", name="sp")
                                nc.tensor.matmul(
                                    sp[:, 0:cw],
                                    kT[kv][:, ki * 128:(ki + 1) * 128],
                                    qT[h][:, q0:q0 + cw],
                                    start=True, stop=True)
                                if causal:
                                    if q0 == ki * 128:
                                        nc.tensor.matmul(
                                            sp[:, 0:128], ident_bf[:],
                                            mask_t_sb[:, ki * 128:(ki + 1) * 128],
                                            start=False, stop=True,
                                            skip_group_check=True)
                                else:
                                    mt = maskp.tile([128, QC], BF16, tag="mask_t",
                                                    name="mt")
                                    nc.sync.dma_start(
                                        mt[:, 0:cw],
                                        mask_t_in.ap()[ki * 128:(ki + 1) * 128,
                                                       q0:q0 + cw])
                                    nc.tensor.matmul(
                                        sp[:, 0:cw], ident_bf[:], mt[:, 0:cw],
                                        start=False, stop=True,
                                        skip_group_check=True)
                                sp_of[t] = sp

                            def emit_consume(t):
                                h, qc, ki, q0, cw, first, last = tiles[t]
                                kv = h // N_REP
                                qc0 = qc * QC
                                if first:
                                    ctx["ot"] = psot.tile([128, QC], F32, tag="ot",
                                                          name="ot")
                                    ctx["l"] = psl.tile([128, QC], F32, tag="l",
                                                        name="l")
                                sp = sp_of[t]
                                sp_of[t] = None
                                pt = ptp.tile([128, QC], BF16, tag="pt", name="pt")
                                nc.scalar.activation(pt[:, 0:cw], sp[:, 0:cw],
                                                     AF.Exp, bias=nbias[:],
                                                     scale=1.0)
                                off = q0 - qc0
                                nc.tensor.matmul(
                                    ctx["ot"][:, off:off + cw],
                                    v_sb[kv][:, ki * 128:(ki + 1) * 128],
                                    pt[:, 0:cw],
                                    start=first, stop=last,
                                    skip_group_check=True)
                                nc.tensor.matmul(
                                    ctx["l"][:, off:off + cw],
                                    ones_bf[:],
                                    pt[:, 0:cw],
                                    start=first, stop=last,
                                    skip_group_check=True)
                                if last:
                                    lnl = scrp.tile([128, QC], F32, tag="lnl",
                                                    name="lnl")
                                    nc.scalar.activation(lnl[:], ctx["l"][:], AF.Ln)
                                    linv = linvp.tile([128, QC], F32, tag="linv",
                                                      name="linv")
                                    nc.scalar.activation(linv[:], lnl[:], AF.Exp,
                                                         scale=-1.0)
                                    nc.vector.tensor_mul(
                                        oT_sb[h][:, qc0:qc0 + QC],
                                        ctx["ot"][:], linv[:])

                            LOOKAHEAD = 2
                            for t in range(min(LOOKAHEAD, n_tiles)):
                                emit_score(t)
                            for t in range(n_tiles):
                                if t + LOOKAHEAD < n_tiles:
                                    emit_score(t + LOOKAHEAD)
                                emit_consume(t)

                        # ---------- Phase 3: output projection ----------
                        with (
                            tc.tile_pool(name="wop", bufs=9) as wop,
                            tc.tile_pool(name="outp", bufs=3) as outp,
                            tc.tile_pool(name="ps_out", bufs=2, space="PSUM") as psout,
                        ):
                            for half in range(2):
                                wo_tiles = []
                                for h in range(NH):
                                    w = wop.tile([128, 2048], BF16, tag="w",
                                                 name="wotile")
                                    nc.sync.dma_start(
                                        w[:], wo.ap()[h * HD:(h + 1) * HD,
                                                      half * 2048:(half + 1) * 2048])
                                    wo_tiles.append(w)
                                for tt in range(NQT):
                                    po = psout.tile([128, 2048], F32, tag="po",
                                                    name="po")
                                    for h in range(NH):
                                        lhsT = oT_sb[h][:, tt * 128:(tt + 1) * 128]
                                        for dj in range(4):
                                            nc.tensor.matmul(
                                                po[:, dj * 512:(dj + 1) * 512],
                                                lhsT,
                                                wo_tiles[h][:, dj * 512:(dj + 1) * 512],
                                                start=(h == 0), stop=(h == NH - 1),
                                                skip_group_check=True)
                                    osb = outp.tile([128, 2048], BF16, tag="osb",
                                                    name="osb")
                                    nc.scalar.copy(osb[:], po[:])
                                    nc.sync.dma_start(
                                        out.ap()[tt * 128:(tt + 1) * 128,
                                                 half * 2048:(half + 1) * 2048],
                                        osb[:])

    nc.compile()
    return nc


def _is_causal(mask: np.ndarray) -> bool:
    if mask.shape != (S, S):
        return False
    neg = mask[0, 1]
    if not (neg <= -1e8):
        return False
    expect = np.triu(np.full((S, S), neg, dtype=np.float32), 1)
    return np.array_equal(mask, expect)


def _est_bias(x, wq, wk) -> float:
    """Upper-ish bound on attention scores (post 1/sqrt(HD) scale), from a
    strided token sample.  Softmax uses exp(s - bias): ratio-exact for any
    bias; bias only needs to keep exp() in fp32/bf16 range."""
    idx = np.arange(0, S, S // 128)
    xs = x[:, idx, :].astype(np.float32)                     # [B,128,D]
    q = (xs @ (wq.astype(np.float32) / np.sqrt(HD))).reshape(B, len(idx), H, HD)
    k = (xs @ wk.astype(np.float32)).reshape(B, len(idx), KVH, HD)
    k = np.repeat(k, N_REP, axis=2)
    s = np.einsum('bihd,bjhd->bhij', q, k)
    est = float(np.ceil(s.max() + 6.0 * s.std() + 4.0))
    return float(np.clip(est, 4.0, 60.0))


_PROG = {}


def _get_prog(causal: bool, repeat: int = 1, bias: float = 16.0):
    key = (causal, repeat, float(bias))
    if key not in _PROG:
        _PROG[key] = _build(causal, repeat, bias)
    return _PROG[key]


def _stage(x, cos, sin, mask, wq, wk, wv, wo, causal):
    perm = np.concatenate([np.arange(0, HD, 2), np.arange(1, HD, 2)])
    # fold the 1/sqrt(HD) attention scale into wq (RoPE is linear in q)
    wq_p = (wq * np.float32(1.0 / np.sqrt(HD))).reshape(D, H, HD)[:, :, perm]
    wk_p = wk.reshape(D, KVH, HD)[:, :, perm]
    wv_r = wv.reshape(D, KVH, HD)

    cos_rep = np.ascontiguousarray(
        np.concatenate([cos.T, cos.T], axis=0), dtype=np.float32)
    sin_rep = np.ascontiguousarray(
        np.concatenate([sin.T, sin.T], axis=0), dtype=np.float32)

    if causal:
        mask_diag_t = np.empty((128, S), dtype=np.float32)
        for qi in range(NQT):
            blk = mask[qi * 128:(qi + 1) * 128, qi * 128:(qi + 1) * 128]
            mask_diag_t[:, qi * 128:(qi + 1) * 128] = blk.T
        mask_diag_t = mask_diag_t.astype(ml_dtypes.bfloat16)
    else:
        mask_full_t = np.ascontiguousarray(mask.T, dtype=np.float32).astype(
            ml_dtypes.bfloat16)

    xT = [
        np.ascontiguousarray(x[b].T).astype(ml_dtypes.bfloat16) for b in range(B)
    ]

    def tile_layout(w, ncols):
        # [D, ncols*128] -> [ncols*128, DCH*128] slabs:
        # slab[c*128+p, d*128+j] = w[d*128+p, c*128+j]
        return np.ascontiguousarray(
            w.reshape(DCH, 128, ncols, 128).transpose(2, 1, 0, 3)
            .reshape(ncols * 128, DCH * 128))

    in_maps = []
    for c in range(N_CORES):
        b, g = c // 4, c % 4
        m = {
            "xT": xT[b],
            "wq": tile_layout(
                wq_p[:, 8 * g:8 * g + 8].reshape(D, NH * HD), NH
            ).astype(ml_dtypes.bfloat16),
            "wk": tile_layout(
                wk_p[:, 2 * g:2 * g + 2].reshape(D, NKV * HD), NKV
            ).astype(ml_dtypes.bfloat16),
            "wv": tile_layout(
                wv_r[:, 2 * g:2 * g + 2].reshape(D, NKV * HD), NKV
            ).astype(ml_dtypes.bfloat16),
            "wo": np.ascontiguousarray(
                wo[1024 * g:1024 * (g + 1), :]).astype(ml_dtypes.bfloat16),
            "cos_rep": cos_rep,
            "sin_rep": sin_rep,
        }
        if causal:
            m["mask_diag_t"] = mask_diag_t
        else:
            m["mask_full_t"] = mask_full_t
        in_maps.append(m)
    return in_maps


def _run(inputs, trace=False):
    x = np.asarray(inputs["x"], dtype=np.float32)
    cos = np.asarray(inputs["cos"], dtype=np.float32)
    sin = np.asarray(inputs["sin"], dtype=np.float32)
    mask = np.asarray(inputs["mask"], dtype=np.float32)
    wq = np.asarray(inputs["wq"], dtype=np.float32)
    wk = np.asarray(inputs["wk"], dtype=np.float32)
    wv = np.asarray(inputs["wv"], dtype=np.float32)
    wo = np.asarray(inputs["wo"], dtype=np.float32)

    causal = _is_causal(mask)
    bias = _est_bias(x, wq, wk)
    nc = _get_prog(causal, 1, bias)
    in_maps = _stage(x, cos, sin, mask, wq, wk, wv, wo, causal)
    res = run_bass_kernel_spmd(nc, in_maps, list(range(N_CORES)), trace=trace)

    out = np.empty((B, S, D), dtype=np.float32)
    for b in range(B):
        acc = res.results[4 * b]["out"].astype(np.float32)
        for g in range(1, 4):
            acc = acc + res.results[4 * b + g]["out"].astype(np.float32)
        out[b] = acc
    return out, res


def kernel(**inputs) -> np.ndarray:
    out, _ = _run(inputs, trace=False)
    return out


# revision 15
# speedup vs baseline: 1.3975x; 1.3975x over previous
"""GQA attention block (B=2,S=2048,D=4096,H=32,KVH=8,HD=128) on 8 trn2 cores.

Sharding: core c -> batch b=c//4, head-group g=c%4 (8 q heads, 2 kv heads per
core).  Each core computes QKV projections + RoPE + causal attention + its
slice of the output projection; the host sums the 4 partial outputs per batch.

Attention uses a max-free softmax: p = exp(s - B) with a data-calibrated
constant bias B (exact softmax ratio invariance), so the per-row max/stats
pass is eliminated.  The denominator l = sum_k p is accumulated on the PE
with a broadcast ones-matmul, inverted as exp(-ln l) on the Act engine, and
folded into the o^T eviction on the vector engine.
"""

import numpy as np
import ml_dtypes

import concourse.bass as bass
import concourse.tile as tile
import concourse.mybir as mybir
from concourse import bacc
from concourse.bass_utils import run_bass_kernel_spmd
from concourse.masks import make_identity

F32 = mybir.dt.float32
F32R = mybir.dt.float32r
BF16 = mybir.dt.bfloat16
AX = mybir.AxisListType
AF = mybir.ActivationFunctionType

B, S, D = 2, 2048, 4096
H, KVH, HD = 32, 8, 128
N_REP = H // KVH
N_CORES = 8
NH = 8            # q heads per core
NKV = 2           # kv heads per core
DCH = D // 128    # contraction chunks
NQT = S // 128    # q tiles
NKT = S // 128    # k tiles
NCT = NH + 2 * NKV  # projection col-tiles: 8 q, 2 k, 2 v
QC = 512          # attention q-chunk width
NQC = S // QC


def _build(causal: bool, repeat: int = 1, bias: float = 16.0):
    nc = bacc.Bacc(None, target_bir_lowering=False, debug=False)

    TPW = 512                 # qkv token-pass width
    NTPW = S // TPW

    xT = nc.dram_tensor("xT", [D, S], BF16, kind="ExternalInput")
    # weights staged as per-col-tile slabs: slab[c*128+p, d*128+j] = w[d*128+p, c*128+j]
    # -> one contiguous-line DMA loads all 32 contraction tiles of col-tile c
    wq = nc.dram_tensor("wq", [NH * 128, DCH * 128], BF16, kind="ExternalInput")
    wk = nc.dram_tensor("wk", [NKV * 128, DCH * 128], BF16, kind="ExternalInput")
    wv = nc.dram_tensor("wv", [NKV * 128, DCH * 128], BF16, kind="ExternalInput")
    wo = nc.dram_tensor("wo", [NH * HD, D], BF16, kind="ExternalInput")
    cos_rep = nc.dram_tensor("cos_rep", [128, S], F32, kind="ExternalInput")
    sin_rep = nc.dram_tensor("sin_rep", [128, S], F32, kind="ExternalInput")
    if causal:
        mask_t_in = nc.dram_tensor("mask_diag_t", [128, S], BF16, kind="ExternalInput")
    else:
        mask_t_in = nc.dram_tensor("mask_full_t", [S, S], BF16, kind="ExternalInput")
    out = nc.dram_tensor("out", [S, D], BF16, kind="ExternalOutput")

    with tile.TileContext(nc) as tc:
        with (
            tc.tile_pool(name="const", bufs=1) as constp,
        ):
            ident_f = constp.tile([128, 128], F32, tag="ident_f")
            make_identity(nc, ident_f[:])
            ident_r = constp.tile([128, 128], F32R, tag="ident_r")
            nc.vector.tensor_copy(ident_r[:], ident_f[:])
            ident_bf = constp.tile([128, 128], BF16, tag="ident_bf")
            nc.vector.tensor_copy(ident_bf[:], ident_f[:])
            ones_bf = constp.tile([128, 128], BF16, tag="ones_bf")
            nc.vector.memset(ones_bf[:], 1.0)
            nbias = constp.tile([128, 1], F32, tag="nbias")
            nc.vector.memset(nbias[:], -float(bias))

            for _rep in range(repeat):
                with (
                    tc.tile_pool(name="acts", bufs=1) as acts,
                ):
                    qT = [acts.tile([128, S], F32R, tag=f"qT{h}", name=f"qT{h}")
                          for h in range(NH)]
                    kT = [acts.tile([128, S], F32R, tag=f"kT{k}", name=f"kT{k}")
                          for k in range(NKV)]
                    v_sb = [acts.tile([128, S], BF16, tag=f"v{k}", name=f"v{k}")
                            for k in range(NKV)]

                    # ---------- Phase 1: QKV projection + RoPE ----------
                    with (
                        tc.tile_pool(name="xq", bufs=DCH + 16) as xqp,
                        tc.tile_pool(name="wslab", bufs=4) as wslabp,
                        tc.tile_pool(name="rope", bufs=2) as ropep,
                        tc.tile_pool(name="trig", bufs=1) as trigp,
                        tc.tile_pool(name="vtmp", bufs=2) as vtmpp,
                        tc.tile_pool(name="ps_qkv", bufs=2, space="PSUM") as psq,
                        tc.tile_pool(name="ps_v", bufs=2, space="PSUM") as psv,
                    ):
                        def rope_evict(ps, dest, cos_t, sin_t):
                            qc = ropep.tile([128, TPW], F32, tag="qc", name="qc")
                            qs = ropep.tile([128, TPW], F32, tag="qs", name="qs")
                            qsw = ropep.tile([128, TPW], F32, tag="qsw", name="qsw")
                            nc.vector.tensor_mul(qc[:], ps[:], cos_t[:])
                            nc.vector.tensor_mul(qs[:], ps[:], sin_t[:])
                            nc.scalar.dma_start(qsw[0:64, :], qs[64:128, :])
                            nc.scalar.dma_start(qsw[64:128, :], qs[0:64, :])
                            nc.vector.tensor_sub(dest[0:64, :], qc[0:64, :], qsw[0:64, :])
                            nc.vector.tensor_add(dest[64:128, :], qc[64:128, :], qsw[64:128, :])

                        # trig loads go on the gpsimd queue so they don't
                        # head-of-line block the first weight slab on sync
                        cos_full = trigp.tile([128, S], F32, tag="cos", name="cos_full")
                        sin_full = trigp.tile([128, S], F32, tag="sin", name="sin_full")
                        nc.gpsimd.dma_start(cos_full[:], cos_rep.ap())
                        nc.gpsimd.dma_start(sin_full[:], sin_rep.ap())
                        for tp in range(NTPW):
                            t0 = tp * TPW
                            cos_t = cos_full[:, t0:t0 + TPW]
                            sin_t = sin_full[:, t0:t0 + TPW]
                            xt = []
                            for d in range(DCH):
                                xd = xqp.tile([128, TPW], BF16, tag="x", name="xd")
                                # alternate queues so x streaming keeps up
                                # with the PE during the first col-tile
                                q = nc.scalar if d % 2 == 0 else nc.gpsimd
                                q.dma_start(
                                    xd[:], xT.ap()[d * 128:(d + 1) * 128, t0:t0 + TPW])
                                xt.append(xd)

                            for ct in list(range(NH, NCT)) + list(range(NH)):
                                if ct < NH:
                                    wsrc, col, is_q, is_rope = wq, ct, True, True
                                elif ct < NH + NKV:
                                    k = ct - NH
                                    wsrc, col, is_q, is_rope = wk, k, False, True
                                else:
                                    k = ct - NH - NKV
                                    wsrc, col, is_q, is_rope = wv, k, False, False

                                slab = wslabp.tile([128, DCH * 128], BF16,
                                                   tag="wslab", name="slab")
                                nc.sync.dma_start(
                                    slab[:], wsrc.ap()[col * 128:(col + 1) * 128, :])
                                ps = psq.tile([128, TPW], F32, tag="ps", name="ps")
                                for d in range(DCH):
                                    nc.tensor.matmul(ps[:], slab[:, d * 128:(d + 1) * 128],
                                                     xt[d][:],
                                                     start=(d == 0), stop=(d == DCH - 1))

                                if is_q:
                                    rope_evict(ps, qT[ct][:, t0:t0 + TPW], cos_t, sin_t)
                                elif is_rope:
                                    rope_evict(ps, kT[k][:, t0:t0 + TPW], cos_t, sin_t)
                                else:
                                    vt = vtmpp.tile([128, TPW], F32R, tag="vt", name="vt")
                                    nc.scalar.copy(vt[:], ps[:])
                                    for kk in range(TPW // 128):
                                        tt = (t0 + kk * 128) // 128
                                        pv = psv.tile([128, 128], F32R, tag="pv", name="pv")
                                        nc.tensor.transpose(
                                            pv[:], vt[:, kk * 128:(kk + 1) * 128], ident_r[:])
                                        nc.scalar.copy(
                                            v_sb[k][:, tt * 128:(tt + 1) * 128], pv[:])

                    # ---------- Phases 2+3 share the oT + wo pools ----------
                    with (
                        tc.tile_pool(name="oTp", bufs=1) as oTp,
                        tc.tile_pool(name="wop", bufs=1) as wop,
                    ):
                        oT_sb = [oTp.tile([128, S], BF16, tag=f"oT{h}",
                                          name=f"oT{h}") for h in range(NH)]
                        # preload all wo tiles now: distinct SBUF from the
                        # phase-2 pools, so the DMAs run during attention
                        wo_tiles = []
                        for half in range(2):
                            row = []
                            for h in range(NH):
                                w = wop.tile([128, 2048], BF16,
                                             tag=f"w{half}_{h}", name="wotile")
                                nc.sync.dma_start(
                                    w[:], wo.ap()[h * HD:(h + 1) * HD,
                                                  half * 2048:(half + 1) * 2048])
                                row.append(w)
                            wo_tiles.append(row)

                        # ---------- Phase 2: attention ----------
                        with (
                            tc.tile_pool(name="maskp", bufs=1 if causal else 4) as maskp,
                            tc.tile_pool(name="ptp", bufs=3) as ptp,
                            tc.tile_pool(name="scrp", bufs=2) as scrp,
                            tc.tile_pool(name="linvp", bufs=2) as linvp,
                            tc.tile_pool(name="ps_s", bufs=3, space="PSUM") as pss,
                            tc.tile_pool(name="ps_ot", bufs=2, space="PSUM") as psot,
                            tc.tile_pool(name="ps_l", bufs=2, space="PSUM") as psl,
                        ):
                            if causal:
                                mask_t_sb = maskp.tile([128, S], BF16, tag="mask_t")
                                nc.sync.dma_start(mask_t_sb[:], mask_t_in.ap())

                            # flat tile list: (h, qc, ki, q0, cw, first, last)
                            tiles = []
                            for h in range(NH):
                                for qc in range(NQC):
                                    qc0 = qc * QC
                                    nki = (qc0 + QC) // 128 if causal else NKT
                                    for ki in range(nki):
                                        q0 = max(ki * 128, qc0) if causal else qc0
                                        tiles.append(
                                            (h, qc, ki, q0, qc0 + QC - q0,
                                             ki == 0, ki == nki - 1))
                                    # (h, qc) boundary marker
                            n_tiles = len(tiles)
                            sp_of = [None] * n_tiles
                            ctx = {}

                            def emit_score(t):
                                h, qc, ki, q0, cw, first, last = tiles[t]
                                kv = h // N_REP
                                sp = pss.tile([128, QC], F32, tag="sp", name="sp")
                                nc.tensor.matmul(
                                    sp[:, 0:cw],
                                    kT[kv][:, ki * 128:(ki + 1) * 128],
                                    qT[h][:, q0:q0 + cw],
                                    start=True, stop=True)
                                if causal:
                                    if q0 == ki * 128:
                                        nc.tensor.matmul(
                                            sp[:, 0:128], ident_bf[:],
                                            mask_t_sb[:, ki * 128:(ki + 1) * 128],
                                            start=False, stop=True,
                                            skip_group_check=True)
                                else:
                                    mt = maskp.tile([128, QC], BF16, tag="mask_t",
                                                    name="mt")
                                    nc.sync.dma_start(
                                        mt[:, 0:cw],
                                        mask_t_in.ap()[ki * 128:(ki + 1) * 128,
                                                       q0:q0 + cw])
                                    nc.tensor.matmul(
                                        sp[:, 0:cw], ident_bf[:], mt[:, 0:cw],
                                        start=False, stop=True,
                                        skip_group_check=True)
                                sp_of[t] = sp

                            def emit_consume(t):
                                h, qc, ki, q0, cw, first, last = tiles[t]
                                kv = h // N_REP
                                qc0 = qc * QC
                                if first:
                                    ctx["ot"] = psot.tile([128, QC], F32, tag="ot",
                                                          name="ot")
                                    ctx["l"] = psl.tile([128, QC], F32, tag="l",
                                                        name="l")
                                sp = sp_of[t]
                                sp_of[t] = None
                                pt = ptp.tile([128, QC], BF16, tag="pt", name="pt")
                                nc.scalar.activation(pt[:, 0:cw], sp[:, 0:cw],
                                                     AF.Exp, bias=nbias[:],
                                                     scale=1.0)
                                off = q0 - qc0
                                nc.tensor.matmul(
                                    ctx["ot"][:, off:off + cw],
                                    v_sb[kv][:, ki * 128:(ki + 1) * 128],
                                    pt[:, 0:cw],
                                    start=first, stop=last,
                                    skip_group_check=True)
                                nc.tensor.matmul(
                                    ctx["l"][:, off:off + cw],
                                    ones_bf[:],
                                    pt[:, 0:cw],
                                    start=first, stop=last,
                                    skip_group_check=True)
                                if last:
                                    pending.append((t + 3, h, qc0,
                                                    ctx["ot"], ctx["l"]))

                            def emit_evict():
                                _, h, qc0, ot, l = pending.pop(0)
                                linv = linvp.tile([128, QC], F32, tag="linv",
                                                  name="linv")
                                nc.vector.reciprocal(linv[:], l[:])
                                nc.vector.tensor_mul(
                                    oT_sb[h][:, qc0:qc0 + QC], ot[:], linv[:])

                            # evictions are deferred a couple of tiles and run
                            # on the otherwise-idle DVE (reciprocal + scale),
                            # keeping Exp as the only Act table in use
                            pending = []
                            LOOKAHEAD = 2
                            for t in range(min(LOOKAHEAD, n_tiles)):
                                emit_score(t)
                            for t in range(n_tiles):
                                if t + LOOKAHEAD < n_tiles:
                                    emit_score(t + LOOKAHEAD)
                                emit_consume(t)
                                while pending and pending[0][0] <= t:
                                    emit_evict()
                            while pending:
                                emit_evict()

                        # ---------- Phase 3: output projection ----------
                        with (
                            tc.tile_pool(name="outp", bufs=3) as outp,
                            tc.tile_pool(name="ps_out", bufs=2, space="PSUM") as psout,
                        ):
                            for half in range(2):
                                for tt in range(NQT):
                                    po = psout.tile([128, 2048], F32, tag="po",
                                                    name="po")
                                    for h in range(NH):
                                        lhsT = oT_sb[h][:, tt * 128:(tt + 1) * 128]
                                        for dj in range(4):
                                            nc.tensor.matmul(
                                                po[:, dj * 512:(dj + 1) * 512],
                                                lhsT,
                                                wo_tiles[half][h][:, dj * 512:(dj + 1) * 512],
                                                start=(h == 0), stop=(h == NH - 1),
                                                skip_group_check=True)
                                    osb = outp.tile([128, 2048], BF16, tag="osb",
                                                    name="osb")
                                    nc.scalar.copy(osb[:], po[:])
                                    nc.sync.dma_start(
                                        out.ap()[tt * 128:(tt + 1) * 128,
                                                 half * 2048:(half + 1) * 2048],
                                        osb[:])

    nc.compile()
    return nc


def _is_causal(mask: np.ndarray) -> bool:
    if mask.shape != (S, S):
        return False
    neg = mask[0, 1]
    if not (neg <= -1e8):
        return False
    expect = np.triu(np.full((S, S), neg, dtype=np.float32), 1)
    return np.array_equal(mask, expect)


def _est_bias(x, wq, wk) -> float:
    """Upper-ish bound on attention scores (post 1/sqrt(HD) scale), from a
    strided token sample.  Softmax uses exp(s - bias): ratio-exact for any
    bias; bias only needs to keep exp() in fp32/bf16 range."""
    idx = np.arange(0, S, S // 128)
    xs = x[:, idx, :].astype(np.float32)                     # [B,128,D]
    q = (xs @ (wq.astype(np.float32) / np.sqrt(HD))).reshape(B, len(idx), H, HD)
    k = (xs @ wk.astype(np.float32)).reshape(B, len(idx), KVH, HD)
    k = np.repeat(k, N_REP, axis=2)
    s = np.einsum('bihd,bjhd->bhij', q, k)
    est = float(np.ceil(s.max() + 6.0 * s.std() + 4.0))
    return float(np.clip(est, 4.0, 60.0))


_PROG = {}


def _get_prog(causal: bool, repeat: int = 1, bias: float = 16.0):
    key = (causal, repeat, float(bias))
    if key not in _PROG:
        _PROG[key] = _build(causal, repeat, bias)
    return _PROG[key]


def _stage(x, cos, sin, mask, wq, wk, wv, wo, causal):
    perm = np.concatenate([np.arange(0, HD, 2), np.arange(1, HD, 2)])
    # fold the 1/sqrt(HD) attention scale into wq (RoPE is linear in q)
    wq_p = (wq * np.float32(1.0 / np.sqrt(HD))).reshape(D, H, HD)[:, :, perm]
    wk_p = wk.reshape(D, KVH, HD)[:, :, perm]
    wv_r = wv.reshape(D, KVH, HD)

    cos_rep = np.ascontiguousarray(
        np.concatenate([cos.T, cos.T], axis=0), dtype=np.float32)
    sin_rep = np.ascontiguousarray(
        np.concatenate([sin.T, sin.T], axis=0), dtype=np.float32)

    if causal:
        mask_diag_t = np.empty((128, S), dtype=np.float32)
        for qi in range(NQT):
            blk = mask[qi * 128:(qi + 1) * 128, qi * 128:(qi + 1) * 128]
            mask_diag_t[:, qi * 128:(qi + 1) * 128] = blk.T
        mask_diag_t = mask_diag_t.astype(ml_dtypes.bfloat16)
    else:
        mask_full_t = np.ascontiguousarray(mask.T, dtype=np.float32).astype(
            ml_dtypes.bfloat16)

    xT = [
        np.ascontiguousarray(x[b].T).astype(ml_dtypes.bfloat16) for b in range(B)
    ]

    def tile_layout(w, ncols):
        # [D, ncols*128] -> [ncols*128, DCH*128] slabs:
        # slab[c*128+p, d*128+j] = w[d*128+p, c*128+j]
        return np.ascontiguousarray(
            w.reshape(DCH, 128, ncols, 128).transpose(2, 1, 0, 3)
            .reshape(ncols * 128, DCH * 128))

    in_maps = []
    for c in range(N_CORES):
        b, g = c // 4, c % 4
        m = {
            "xT": xT[b],
            "wq": tile_layout(
                wq_p[:, 8 * g:8 * g + 8].reshape(D, NH * HD), NH
            ).astype(ml_dtypes.bfloat16),
            "wk": tile_layout(
                wk_p[:, 2 * g:2 * g + 2].reshape(D, NKV * HD), NKV
            ).astype(ml_dtypes.bfloat16),
            "wv": tile_layout(
                wv_r[:, 2 * g:2 * g + 2].reshape(D, NKV * HD), NKV
            ).astype(ml_dtypes.bfloat16),
            "wo": np.ascontiguousarray(
                wo[1024 * g:1024 * (g + 1), :]).astype(ml_dtypes.bfloat16),
            "cos_rep": cos_rep,
            "sin_rep": sin_rep,
        }
        if causal:
            m["mask_diag_t"] = mask_diag_t
        else:
            m["mask_full_t"] = mask_full_t
        in_maps.append(m)
    return in_maps


def _run(inputs, trace=False):
    x = np.asarray(inputs["x"], dtype=np.float32)
    cos = np.asarray(inputs["cos"], dtype=np.float32)
    sin = np.asarray(inputs["sin"], dtype=np.float32)
    mask = np.asarray(inputs["mask"], dtype=np.float32)
    wq = np.asarray(inputs["wq"], dtype=np.float32)
    wk = np.asarray(inputs["wk"], dtype=np.float32)
    wv = np.asarray(inputs["wv"], dtype=np.float32)
    wo = np.asarray(inputs["wo"], dtype=np.float32)

    causal = _is_causal(mask)
    bias = _est_bias(x, wq, wk)
    nc = _get_prog(causal, 1, bias)
    in_maps = _stage(x, cos, sin, mask, wq, wk, wv, wo, causal)
    res = run_bass_kernel_spmd(nc, in_maps, list(range(N_CORES)), trace=trace)

    out = np.empty((B, S, D), dtype=np.float32)
    for b in range(B):
        acc = res.results[4 * b]["out"].astype(np.float32)
        for g in range(1, 4):
            acc = acc + res.results[4 * b + g]["out"].astype(np.float32)
        out[b] = acc
    return out, res


def kernel(**inputs) -> np.ndarray:
    out, _ = _run(inputs, trace=False)
    return out
